# revision 1
# baseline (speedup 1.0000x reference)
"""Trainium2 Bass kernel for GQA sliding-window attention with logit soft-cap.

Problem: B=2, T=2048, D=3584, N=16 q-heads, K=8 kv-heads, H=256,
sliding window 1024, causal, soft-cap 50, query scale 0.0625, RoPE.

Sharding: 8 cores = 2 (batch) x 4 (head groups). Each core handles one
batch and 4 q-heads / 2 kv-heads (tensor parallel on the head axis of
q_w/kv_w/out_w). Host sums the 4 partial out-projections per batch.

On-device dataflow (all matmuls float32r = tf32-like at full PE rate):
  KV pass:  kT/vT = (x @ W)^T for both kv heads, RoPE on k.
  Q passes: qT per head in two 2-head passes (RoPE + scale fused).
  Attention (transposed, no running max needed thanks to the soft-cap
  bound): logits^T tiles [k,q], p = exp(50*tanh(L)-50), PV + column
  sums via a ones-matmul accumulate in PSUM, normalize with a
  partition-broadcast reciprocal -> encT.
  Out-proj: two 4-chunk passes; second accumulates into `out` via DMA.

Emission is software-pipelined: q-pass B interleaves with attention
group 0 (heads 0-1), out-proj pass A interleaves with attention group 1,
so TensorE keeps dense work while ScalarE runs the softmax chain.
"""

import os
import sys

sys.path.insert(0, "/opt/trn_rl_repo")

import numpy as np

B, T, D = 2, 2048, 3584
NQ, NKV, H = 16, 8, 256
P = 128
DC = D // P                 # 28 contraction chunks
HEADS_PER_CORE = 4
KV_PER_CORE = 2
SOFT_CAP = 50.0
SCALE = 0.0625
WINDOW = 1024
BASE_FREQ = 10000.0
QTILE = 512
NQT = T // QTILE            # 4
NKT = T // P                # 16

_NC_CACHE = {}
LAST_RESULTS = None


def _kt_list(qt):
    """Valid k-tiles for q-block qt with mask index (None = fully allowed)."""
    Q0 = qt * QTILE
    out = []
    for kt in range(NKT):
        K0 = kt * P
        if K0 > Q0 + QTILE - 1:
            continue
        if K0 + P - 1 <= Q0 - WINDOW:
            continue
        rel = K0 - Q0
        if rel >= 0:
            out.append((kt, rel // P))
        else:
            w = Q0 - K0 - WINDOW
            if -QTILE < w <= 0:
                out.append((kt, 4 + (-w) // P))
            else:
                out.append((kt, None))
    return out


def _make_masks():
    m = np.zeros((8, P, QTILE), np.float32)
    i = np.arange(P)[:, None]
    j = np.arange(QTILE)[None, :]
    for r in range(4):           # diag: allowed iff i <= j - rel
        m[r] = np.where(i <= j - r * P, 1.0, 0.0)
    for wi in range(4):          # window: allowed iff i > j + w
        m[4 + wi] = np.where(i > j - wi * P, 1.0, 0.0)
    return m


def _build_nc():
    import concourse.bacc as bacc
    import concourse.mybir as mybir
    import concourse.tile as tile
    from concourse.masks import make_identity

    f32 = mybir.dt.float32
    f32r = mybir.dt.float32r
    AF = mybir.ActivationFunctionType
    MULT = mybir.AluOpType.mult

    nc = bacc.Bacc()
    xT = nc.dram_tensor("xT", (D, T), f32r, kind="ExternalInput")
    qw = nc.dram_tensor("qw", (HEADS_PER_CORE, P, DC, H), f32r, kind="ExternalInput")
    kw = nc.dram_tensor("kw", (KV_PER_CORE, P, DC, H), f32r, kind="ExternalInput")
    vw = nc.dram_tensor("vw", (KV_PER_CORE, P, DC, H), f32r, kind="ExternalInput")
    ow = nc.dram_tensor("ow", (HEADS_PER_CORE, H, D), f32r, kind="ExternalInput")
    rope = nc.dram_tensor("rope", (P, 2, T), f32, kind="ExternalInput")
    msk = nc.dram_tensor("msk", (P, 8, QTILE), mybir.dt.bfloat16, kind="ExternalInput")
    out = nc.dram_tensor("out", (T, D), f32, kind="ExternalOutput")

    xTr = xT.rearrange("(c p) t -> p c t", p=P)
    QS = SCALE / SOFT_CAP

    with tile.TileContext(nc) as tc:
        dpool_cm = tc.tile_pool(name="dram", bufs=1, space="DRAM")
        dpool = dpool_cm.__enter__()
        qT = dpool.tile([HEADS_PER_CORE, P, 2, T], f32r)
        kT = dpool.tile([KV_PER_CORE, P, 2, T], f32r)
        vT = dpool.tile([KV_PER_CORE, P, 2, T], f32r)
        eT = dpool.tile([HEADS_PER_CORE, P, 2, T], f32r)

        # ---------------- P1: projections + RoPE ----------------
        xt_cm = tc.tile_pool(name="xt", bufs=4)
        xpool = xt_cm.__enter__()
        op_cm = tc.tile_pool(name="op", bufs=2)
        opool = op_cm.__enter__()
        w_cm = tc.tile_pool(name="w", bufs=1)
        wpool = w_cm.__enter__()
        rope_cm = tc.tile_pool(name="rope", bufs=1)
        rpool = rope_cm.__enter__()
        ps1_cm = tc.tile_pool(name="ps1", bufs=1, space="PSUM")
        ps1 = ps1_cm.__enter__()

        rope_sb = rpool.tile([P, 2, T], f32)
        nc.sync.dma_start(rope_sb[:], rope[:])
        cos_a = rope_sb[:, 0]
        sin_a = rope_sb[:, 1]

        wq0 = wpool.tile([P, DC, H], f32r, tag="wq0", name="wq0")
        nc.sync.dma_start(wq0[:], qw[0])

        for half in range(2):  # 0: k0,k1,v0,v1   1: q0..q3
            wts = []
            for j in range(4):
                if half == 1 and j == 0:
                    wts.append(wq0)
                    continue
                wt = wpool.tile([P, DC, H], f32r, tag=f"w{j}", name=f"w{j}")
                src = qw[j] if half == 1 else (kw[j] if j < 2 else vw[j - 2])
                nc.sync.dma_start(wt[:], src[:])
                wts.append(wt)
            for n in range(NQT):
                ns = slice(n * QTILE, (n + 1) * QTILE)
                psums = [
                    [
                        ps1.tile([P, QTILE], f32, tag=f"ps{j}{hc}",
                                 name=f"ps{j}{hc}")
                        for hc in range(2)
                    ]
                    for j in range(4)
                ]
                for dp in range(DC // 2):
                    # two D-chunks per DMA halves the sequencer issue load
                    xt = xpool.tile([P, 2, QTILE], f32r, tag="xt", name="xt")
                    nc.sync.dma_start(xt[:], xTr[:, 2 * dp : 2 * dp + 2, ns])
                    for u in range(2):
                        d = 2 * dp + u
                        for j in range(4):
                            for hc in range(2):
                                nc.tensor.matmul(
                                    psums[j][hc][:],
                                    wts[j][:, d, hc * P : (hc + 1) * P],
                                    xt[:, u],
                                    start=(d == 0),
                                    stop=(d == DC - 1),
                                )
                cos_t, sin_t = cos_a[:, ns], sin_a[:, ns]
                for j in range(4):
                    if half == 0 and j >= 2:  # v: copy out on idle ACT
                        for hc in range(2):
                            o = opool.tile([P, QTILE], f32r, tag=f"o{hc}",
                                           name="o")
                            nc.scalar.copy(o[:], psums[j][hc][:])
                            nc.sync.dma_start(vT[j - 2, :, hc, ns], o[:])
                        continue
                    c0 = opool.tile([P, QTILE], f32, tag="c0", name="c0")
                    s0 = opool.tile([P, QTILE], f32, tag="s0", name="s0")
                    c1 = opool.tile([P, QTILE], f32, tag="c1", name="c1")
                    s1 = opool.tile([P, QTILE], f32, tag="s1", name="s1")
                    o0 = opool.tile([P, QTILE], f32r, tag="o0", name="o0")
                    o1 = opool.tile([P, QTILE], f32r, tag="o1", name="o1")
                    p0, p1 = psums[j][0][:], psums[j][1][:]
                    if half == 1:  # q: fold SCALE/SOFT_CAP into the rotation
                        nc.vector.scalar_tensor_tensor(c0[:], p0, QS, cos_t, MULT, MULT)
                        nc.vector.scalar_tensor_tensor(s0[:], p0, QS, sin_t, MULT, MULT)
                        nc.vector.scalar_tensor_tensor(c1[:], p1, QS, cos_t, MULT, MULT)
                        nc.vector.scalar_tensor_tensor(s1[:], p1, QS, sin_t, MULT, MULT)
                    else:
                        nc.vector.tensor_mul(c0[:], p0, cos_t)
                        nc.vector.tensor_mul(s0[:], p0, sin_t)
                        nc.vector.tensor_mul(c1[:], p1, cos_t)
                        nc.vector.tensor_mul(s1[:], p1, sin_t)
                    nc.vector.tensor_sub(o0[:], c0[:], s1[:])
                    nc.vector.tensor_add(o1[:], c1[:], s0[:])
                    dstT = qT[j] if half == 1 else kT[j]
                    nc.sync.dma_start(dstT[:, 0, ns], o0[:])
                    nc.sync.dma_start(dstT[:, 1, ns], o1[:])

        ps1_cm.__exit__(None, None, None)
        rope_cm.__exit__(None, None, None)
        w_cm.__exit__(None, None, None)
        op_cm.__exit__(None, None, None)
        xt_cm.__exit__(None, None, None)

        # ---------------- P2: attention ----------------
        owp_cm = tc.tile_pool(name="owp", bufs=1)
        owp = owp_cm.__enter__()
        ow_sb = []
        for j in range(7):  # prefetch first 7 out-proj chunks during P2
            wt = owp.tile([P, D], f32r, tag=f"owp{j}", name=f"owp{j}")
            nc.sync.dma_start(wt[:], ow[j // 2, (j % 2) * P : (j % 2 + 1) * P, :])
            ow_sb.append(wt)

        cp_cm = tc.tile_pool(name="cp", bufs=1)
        cpool = cp_cm.__enter__()
        kv_cm = tc.tile_pool(name="kvp", bufs=1)
        kvpool = kv_cm.__enter__()
        qq_cm = tc.tile_pool(name="qq", bufs=3)
        qqpool = qq_cm.__enter__()
        sp_cm = tc.tile_pool(name="sp", bufs=2)
        spool = sp_cm.__enter__()
        ep_cm = tc.tile_pool(name="ep", bufs=2)
        epool = ep_cm.__enter__()
        psL_cm = tc.tile_pool(name="psL", bufs=2, space="PSUM")
        psL = psL_cm.__enter__()
        psA_cm = tc.tile_pool(name="psA", bufs=1, space="PSUM")
        psA = psA_cm.__enter__()

        masks_sb = cpool.tile([P, 8, QTILE], mybir.dt.bfloat16)
        nc.sync.dma_start(masks_sb[:], msk[:])
        ones_f = cpool.tile([P, 1], f32)
        nc.vector.memset(ones_f[:], 1.0)
        ones_r = cpool.tile([P, 1], f32r)
        nc.vector.tensor_copy(ones_r[:], ones_f[:])
        bias_m50 = cpool.tile([P, 1], f32)
        nc.vector.memset(bias_m50[:], -SOFT_CAP)
        idf = cpool.tile([P, P], f32)
        make_identity(nc, idf[:])
        idr = cpool.tile([P, P], f32r)
        nc.vector.tensor_copy(idr[:], idf[:])

        for kvh in range(KV_PER_CORE):
            kT_sb = kvpool.tile([P, 2, T], f32r, tag="kT", name="kT_sb")
            nc.sync.dma_start(kT_sb[:], kT[kvh])
            vT_sb = kvpool.tile([P, 2, T], f32r, tag="vT", name="vT_sb")
            nc.sync.dma_start(vT_sb[:], vT[kvh])
            v_all = kvpool.tile([P, NKT, H], f32r, tag="va", name="v_all")
            for kt in range(NKT):
                for hc in range(2):
                    pst = psL.tile([P, P], f32r, tag="L", name="pst")
                    nc.tensor.transpose(
                        pst[:], vT_sb[:, hc, kt * P : (kt + 1) * P], idr[:]
                    )
                    nc.vector.tensor_copy(
                        v_all[:, kt, hc * P : (hc + 1) * P], pst[:]
                    )
            for qh in (2 * kvh, 2 * kvh + 1):
                for qt in range(NQT):
                    qs = slice(qt * QTILE, (qt + 1) * QTILE)
                    qq = qqpool.tile([P, 2, QTILE], f32r, tag="qq", name="qq")
                    nc.sync.dma_start(qq[:], qT[qh][:, :, qs])
                    kts = _kt_list(qt)
                    db = qt % 2
                    enc_ps = [
                        psA.tile([P, QTILE], f32, tag=f"enc{hc}{db}",
                                 name="enc")
                        for hc in range(2)
                    ]
                    s_ps = psA.tile([1, QTILE], f32, tag=f"sums{db}",
                                    name="s_ps")
                    for i, (kt, mi) in enumerate(kts):
                        st, sp = (i == 0), (i == len(kts) - 1)
                        L = psL.tile([P, QTILE], f32, tag="L", name="L")
                        nc.tensor.matmul(
                            L[:], kT_sb[:, 0, kt * P : (kt + 1) * P], qq[:, 0],
                            start=True, stop=False,
                        )
                        nc.tensor.matmul(
                            L[:], kT_sb[:, 1, kt * P : (kt + 1) * P], qq[:, 1],
                            start=False, stop=True,
                        )
                        tt = spool.tile([P, QTILE], f32, tag="t", name="tt")
                        nc.scalar.activation(tt[:], L[:], AF.Tanh)
                        pp = spool.tile([P, QTILE], f32r, tag="p", name="pp")
                        nc.scalar.activation(
                            pp[:], tt[:], AF.Exp, bias=bias_m50[:],
                            scale=SOFT_CAP,
                        )
                        pu = pp[:]
                        if mi is not None:
                            pm = spool.tile([P, QTILE], f32r, tag="pm",
                                            name="pm")
                            nc.vector.tensor_mul(pm[:], pp[:], masks_sb[:, mi])
                            pu = pm[:]
                        nc.tensor.matmul(
                            enc_ps[0][:], v_all[:, kt, 0:P], pu,
                            start=st, stop=sp,
                        )
                        nc.tensor.matmul(
                            enc_ps[1][:], v_all[:, kt, P:H], pu,
                            start=st, stop=sp,
                        )
                        nc.tensor.matmul(
                            s_ps[:], ones_r[:], pu, start=st, stop=sp
                        )
                    rec = spool.tile([1, QTILE], f32, tag="rec", name="rec")
                    nc.vector.reciprocal(rec[:], s_ps[:])
                    rb = spool.tile([P, QTILE], f32, tag="rb", name="rb")
                    nc.gpsimd.partition_broadcast(rb[:], rec[:])
                    for hc in range(2):
                        eo = epool.tile([P, QTILE], f32r, tag=f"eo{hc}",
                                        name="eo")
                        nc.vector.tensor_mul(eo[:], enc_ps[hc][:], rb[:])
                        nc.sync.dma_start(eT[qh, :, hc, qs], eo[:])

        psA_cm.__exit__(None, None, None)
        psL_cm.__exit__(None, None, None)
        ep_cm.__exit__(None, None, None)
        sp_cm.__exit__(None, None, None)
        qq_cm.__exit__(None, None, None)
        kv_cm.__exit__(None, None, None)
        cp_cm.__exit__(None, None, None)

        # ---------------- P3: output projection ----------------
        ow2_cm = tc.tile_pool(name="ow2", bufs=1)
        ow2 = ow2_cm.__enter__()
        et_cm = tc.tile_pool(name="etp", bufs=2)
        etpool = et_cm.__enter__()
        o3_cm = tc.tile_pool(name="o3", bufs=2)
        o3pool = o3_cm.__enter__()
        po_cm = tc.tile_pool(name="po", bufs=3, space="PSUM")
        popool = po_cm.__enter__()

        for j in range(7, 8):
            wt = ow2.tile([P, D], f32r, tag=f"ow2{j}", name=f"ow2{j}")
            nc.sync.dma_start(
                wt[:], ow[j // 2, (j % 2) * P : (j % 2 + 1) * P, :]
            )
            ow_sb.append(wt)

        SPAN = 512
        for tci in range(T // P):
            ts_ = slice(tci * P, (tci + 1) * P)
            if tci % (SPAN // P) == 0:
                sp_ = slice(tci * P, tci * P + SPAN)
                ets = []
                for j in range(8):
                    et = etpool.tile([P, SPAN], f32r, tag=f"et{j}",
                                     name=f"et{j}")
                    nc.sync.dma_start(et[:], eT[j // 2, :, j % 2, sp_])
                    ets.append(et)
            off = (tci % (SPAN // P)) * P
            lhs = [e[:, off : off + P] for e in ets]
            out_sb = o3pool.tile([P, D], f32, tag="osb", name="osb")
            for nn in range(D // QTILE):
                nns = slice(nn * QTILE, (nn + 1) * QTILE)
                po = popool.tile([P, QTILE], f32, tag="po", name="po")
                for j in range(8):
                    nc.tensor.matmul(
                        po[:], lhs[j][:], ow_sb[j][:, nns],
                        start=(j == 0), stop=(j == 7),
                    )
                if nn % 2 == 0:
                    nc.vector.tensor_copy(out_sb[:, nns], po[:])
                else:
                    nc.scalar.copy(out_sb[:, nns], po[:])
            nc.sync.dma_start(out[ts_, :], out_sb[:])

        po_cm.__exit__(None, None, None)
        o3_cm.__exit__(None, None, None)
        et_cm.__exit__(None, None, None)
        ow2_cm.__exit__(None, None, None)
        owp_cm.__exit__(None, None, None)
        dpool_cm.__exit__(None, None, None)

    nc.finalize()
    return nc


def _install_neff_cache():
    """Cache walrus-compiled NEFFs by BIR hash (compiles are minutes-long)."""
    import hashlib
    import shutil

    import concourse.bass2jax as b2j

    if getattr(b2j, "_ant_neff_cache_installed", False):
        return
    orig = b2j.compile_bir_kernel

    def cached(bir_json, tmpdir, neff_name="file.neff"):
        cdir = os.environ.get("NEFF_CACHE_DIR", "/tmp/neff_cache")
        os.makedirs(cdir, exist_ok=True)
        h = hashlib.sha256(bir_json).hexdigest()[:32]
        cpath = os.path.join(cdir, f"{h}.neff")
        if os.path.exists(cpath):
            dst = os.path.join(tmpdir, "sg00")
            os.makedirs(dst, exist_ok=True)
            dstf = os.path.join(dst, neff_name)
            shutil.copyfile(cpath, dstf)
            return dstf
        r = orig(bir_json, tmpdir, neff_name=neff_name)
        try:
            shutil.copyfile(r, cpath)
        except OSError:
            pass
        return r

    b2j.compile_bir_kernel = cached
    b2j._ant_neff_cache_installed = True


def kernel(x, segment_pos, attn_mask, q_w, kv_w, out_w):
    global LAST_RESULTS
    from concourse.bass_utils import run_bass_kernel_spmd

    _install_neff_cache()

    x = np.asarray(x, np.float32)
    segment_pos = np.asarray(segment_pos, np.int32)
    q_w = np.asarray(q_w, np.float32)
    kv_w = np.asarray(kv_w, np.float32)
    out_w = np.asarray(out_w, np.float32)

    # RoPE tables per batch, host layout [P, 2, T]: [cos, sin]
    ropes = []
    for b in range(B):
        pos = segment_pos[b].astype(np.float32)
        fraction = 2.0 * np.arange(P, dtype=np.float32) / H
        timescale = BASE_FREQ**fraction
        ang = pos[None, :] / timescale[:, None]          # [128, T]
        r = np.stack([np.cos(ang), np.sin(ang)]).astype(np.float32)
        ropes.append(np.ascontiguousarray(r.transpose(1, 0, 2)))
    import ml_dtypes
    masks = np.ascontiguousarray(
        _make_masks().transpose(1, 0, 2).astype(ml_dtypes.bfloat16)
    )

    def _wlayout(w):
        # [nh, D, H] -> [nh, P, DC, H]: per-partition contiguous spans
        return np.ascontiguousarray(
            w.reshape(-1, DC, P, H).transpose(0, 2, 1, 3)
        )

    key = "main"
    if key not in _NC_CACHE:
        _NC_CACHE[key] = _build_nc()
    nc = _NC_CACHE[key]

    in_maps = []
    for core in range(8):
        b, g = core // 4, core % 4
        in_maps.append(
            {
                "xT": np.ascontiguousarray(x[b].T),
                "qw": _wlayout(q_w[4 * g : 4 * g + 4]),
                "kw": _wlayout(kv_w[0, 2 * g : 2 * g + 2]),
                "vw": _wlayout(kv_w[1, 2 * g : 2 * g + 2]),
                "ow": np.ascontiguousarray(out_w[4 * g : 4 * g + 4]),
                "rope": ropes[b],
                "msk": masks,
            }
        )

    res = run_bass_kernel_spmd(nc, in_maps, core_ids=list(range(8)))
    LAST_RESULTS = res

    outv = np.zeros((B, T, D), np.float32)
    for core in range(8):
        outv[core // 4] += res.results[core]["out"]
    return outv



# revision 5
# speedup vs baseline: 1.4925x; 1.4925x over previous
"""Trainium2 Bass kernel for GQA sliding-window attention with logit soft-cap.

Problem: B=2, T=2048, D=3584, N=16 q-heads, K=8 kv-heads, H=256,
sliding window 1024, causal, soft-cap 50, query scale 0.0625, RoPE.

Sharding: 8 cores = 2 (batch) x 4 (head groups). Each core handles one
batch and 4 q-heads / 2 kv-heads. Host sums the 4 partial
out-projections per batch.

v2 design (vs the DRAM-roundtrip baseline):
  - All matmul operands fp16: the f32r LDWEIGHTS (224ns) paced every
    matmul to a 272ns cadence; fp16 weight loads are cheaper and all
    DMA traffic halves.
  - q/k/v/e intermediates live entirely in SBUF (no DRAM roundtrips)
    and V is projected directly into [kpos, h] layout by swapping the
    matmul operands for the V group (no PE transposes).
  - Pass A projects K and V for all T; pass B projects Q per 512-block
    with the previous block's attention tiles woven into the emission
    so ACT softmax work hides under projection matmuls; the final
    block's attention is woven with the output projection.
  - Softmax: p = exp(50*tanh(L*0.0625/50) - 4) in fp16 (the -4 bias
    keeps p in fp16 normal range; it cancels in normalization).
"""

import os
import sys

sys.path.insert(0, "/opt/trn_rl_repo")

import numpy as np

B, T, D = 2, 2048, 3584
NQ, NKV, H = 16, 8, 256
P = 128
DC = D // P                 # 28 contraction chunks
HEADS_PER_CORE = 4
KV_PER_CORE = 2
SOFT_CAP = 50.0
SCALE = 0.0625
WINDOW = 1024
BASE_FREQ = 10000.0
QTILE = 512
NQT = T // QTILE            # 4
NKT = T // P                # 16
EXP_BIAS = -4.0

_NC_CACHE = {}
LAST_RESULTS = None


def _kt_list(qt):
    """Valid k-tiles for q-block qt with mask index (None = fully allowed)."""
    Q0 = qt * QTILE
    out = []
    for kt in range(NKT):
        K0 = kt * P
        if K0 > Q0 + QTILE - 1:
            continue
        if K0 + P - 1 <= Q0 - WINDOW:
            continue
        rel = K0 - Q0
        if rel >= 0:
            out.append((kt, rel // P))
        else:
            w = Q0 - K0 - WINDOW
            if -QTILE < w <= 0:
                out.append((kt, 4 + (-w) // P))
            else:
                out.append((kt, None))
    return out


def _make_masks():
    m = np.zeros((8, P, QTILE), np.float32)
    i = np.arange(P)[:, None]
    j = np.arange(QTILE)[None, :]
    for r in range(4):           # diag: allowed iff i <= j - rel
        m[r] = np.where(i <= j - r * P, 1.0, 0.0)
    for wi in range(4):          # window: allowed iff i > j - wi*P
        m[4 + wi] = np.where(i > j - wi * P, 1.0, 0.0)
    return m


def _build_nc():
    import concourse.bacc as bacc
    import concourse.mybir as mybir
    import concourse.tile as tile

    f32 = mybir.dt.float32
    f16 = mybir.dt.float16
    AF = mybir.ActivationFunctionType

    nc = bacc.Bacc()
    xT = nc.dram_tensor("xT", (D, T), f16, kind="ExternalInput")
    qw = nc.dram_tensor("qw", (HEADS_PER_CORE, P, DC, H), f16,
                        kind="ExternalInput")
    kw = nc.dram_tensor("kw", (KV_PER_CORE, P, DC, H), f16,
                        kind="ExternalInput")
    vw = nc.dram_tensor("vw", (P, DC, 2 * H), f16, kind="ExternalInput")
    ow = nc.dram_tensor("ow", (2 * HEADS_PER_CORE, P, D), f16,
                        kind="ExternalInput")
    rope = nc.dram_tensor("rope", (P, 2, T), f16, kind="ExternalInput")
    msk = nc.dram_tensor("msk", (P, 8, QTILE), f16, kind="ExternalInput")
    out = nc.dram_tensor("out", (T, D), f16, kind="ExternalOutput")

    xTr = xT.rearrange("(c p) t -> p c t", p=P)
    QS = SCALE / SOFT_CAP

    with tile.TileContext(nc) as tc:
        # ------------ persistent SBUF state (~96KB/partition) ------------
        per_cm = tc.tile_pool(name="per", bufs=1)
        per = per_cm.__enter__()
        ones16 = per.tile([P, 1], f16, tag="ones", name="ones16")
        bias_e = per.tile([P, 1], f32, tag="biase", name="bias_e")
        rope_sb = per.tile([P, 2, T], f16, tag="rope", name="rope_sb")
        masks_sb = per.tile([P, 8, QTILE], f16, tag="msk", name="masks_sb")
        cos_a = rope_sb[:, 0]
        sin_a = rope_sb[:, 1]
        kT = [per.tile([P, 2, T], f16, tag=f"kT{kvh}", name=f"kT{kvh}")
              for kvh in range(KV_PER_CORE)]
        v_all = per.tile([P, NKT, 2 * H], f16, tag="vall", name="v_all")
        qT = [[per.tile([P, 2, QTILE], f16, tag=f"qT{par}{qh}",
                        name=f"qT{par}{qh}")
               for qh in range(HEADS_PER_CORE)] for par in range(2)]
        eT = [per.tile([P, 2, T], f16, tag=f"eT{qh}", name=f"eT{qh}")
              for qh in range(HEADS_PER_CORE)]
        nc.vector.memset(ones16[:], 1.0)
        nc.vector.memset(bias_e[:], EXP_BIAS)

        # ------------ pools live across passes A+B ------------
        xp_cm = tc.tile_pool(name="xp", bufs=1)        # 28KB
        xp = xp_cm.__enter__()
        rp_cm = tc.tile_pool(name="rp", bufs=1)        # 8KB
        rp = rp_cm.__enter__()

        def load_x(n, first=False):
            ns = slice(n * QTILE, (n + 1) * QTILE)
            xts = []
            for dp in range(DC // 2):
                xt = xp.tile([P, 2, QTILE], f16, tag=f"xt{dp}",
                             name=f"xt{dp}")
                xts.append(xt)
            order = list(range(DC // 2))
            for dp in order:
                nc.sync.dma_start(xts[dp][:], xTr[:, 2 * dp: 2 * dp + 2, ns])
                if first and dp == 6:
                    nc.sync.dma_start(wk_sb[1][:], kw[1])
            return xts

        def rope_out(p0, p1, ns, dst0, dst1):
            cos_t, sin_t = cos_a[:, ns], sin_a[:, ns]
            c0 = rp.tile([P, QTILE], f32, tag="c0", name="c0")
            s0 = rp.tile([P, QTILE], f32, tag="s0", name="s0")
            c1 = rp.tile([P, QTILE], f32, tag="c1", name="c1")
            s1 = rp.tile([P, QTILE], f32, tag="s1", name="s1")
            nc.vector.tensor_mul(c0[:], p0, cos_t)
            nc.vector.tensor_mul(s0[:], p0, sin_t)
            nc.vector.tensor_mul(c1[:], p1, cos_t)
            nc.vector.tensor_mul(s1[:], p1, sin_t)
            nc.vector.tensor_sub(dst0, c0[:], s1[:])
            nc.vector.tensor_add(dst1, c1[:], s0[:])

        # ------------ pass A: K/V projections ------------
        wkv_cm = tc.tile_pool(name="wkv", bufs=1)      # 56KB
        wkv = wkv_cm.__enter__()
        wk_sb = [wkv.tile([P, DC, H], f16, tag=f"wk{j}", name=f"wk{j}")
                 for j in range(KV_PER_CORE)]
        wv_sb = wkv.tile([P, DC, 2 * H], f16, tag="wv", name="wv_sb")
        psA_cm = tc.tile_pool(name="psA", bufs=2, space="PSUM")
        psA = psA_cm.__enter__()

        nc.sync.dma_start(wk_sb[0][:], kw[0])
        for n in range(NQT):
            ns = slice(n * QTILE, (n + 1) * QTILE)
            xts = load_x(n, first=(n == 0))
            if n == 0:
                nc.sync.dma_start(wv_sb[:], vw[:])
                nc.sync.dma_start(rope_sb[:], rope[:])
                nc.sync.dma_start(masks_sb[:], msk[:])
            for kvh in range(KV_PER_CORE):
                kp = [psA.tile([P, QTILE], f32, tag=f"kp{hc}",
                               name=f"kp{hc}") for hc in range(2)]
                for d in range(DC):
                    xt = xts[d // 2][:, d % 2]
                    for hc in range(2):
                        nc.tensor.matmul(
                            kp[hc][:], wk_sb[kvh][:, d, hc * P:(hc + 1) * P],
                            xt, start=(d == 0), stop=(d == DC - 1))
                rope_out(kp[0][:], kp[1][:], ns,
                         kT[kvh][:, 0, ns], kT[kvh][:, 1, ns])
            for tc_ in range(QTILE // P):
                vp = psA.tile([P, 2 * H], f32, tag="vp", name="vp")
                for d in range(DC):
                    nc.tensor.matmul(
                        vp[:],
                        xts[d // 2][:, d % 2, tc_ * P:(tc_ + 1) * P],
                        wv_sb[:, d, :], start=(d == 0), stop=(d == DC - 1))
                nc.scalar.copy(v_all[:, 4 * n + tc_, :], vp[:])

        psA_cm.__exit__(None, None, None)
        wkv_cm.__exit__(None, None, None)

        # ------------ pass B: Q proj + woven attention ------------
        sp_cm = tc.tile_pool(name="sp", bufs=2)        # ~12KB
        spool = sp_cm.__enter__()
        wq_cm = tc.tile_pool(name="wq", bufs=1)        # 56KB
        wqp = wq_cm.__enter__()
        wq_sb = [wqp.tile([P, DC, H], f16, tag=f"wq{j}", name=f"wq{j}")
                 for j in range(HEADS_PER_CORE)]
        for j in range(HEADS_PER_CORE):
            nc.sync.dma_start(wq_sb[j][:], qw[j])

        psB_cm = tc.tile_pool(name="psB", bufs=1, space="PSUM")   # 3 banks
        psB = psB_cm.__enter__()
        psL_cm = tc.tile_pool(name="psL", bufs=2, space="PSUM")   # 2 banks
        psL = psL_cm.__enter__()
        psQ_cm = tc.tile_pool(name="psQ", bufs=1, space="PSUM")   # 2 banks
        psQ = psQ_cm.__enter__()

        def attn_units(qt):
            """Per-tile emission closures for attention on q-block qt."""
            units = []
            par = qt % 2
            for qh in range(HEADS_PER_CORE):
                kvh = qh // 2
                kts = _kt_list(qt)
                enc = [psB.tile([P, QTILE], f32, tag=f"enc{hc}",
                                name=f"enc{hc}") for hc in range(2)]
                s_ps = psB.tile([1, QTILE], f32, tag="sums", name="s_ps")

                def mk_tile(qh, kvh, enc, s_ps, kt, mi, st, sp):
                    def emit():
                        qq = qT[par][qh]
                        L = psL.tile([P, QTILE], f32, tag="L", name="L")
                        ks = slice(kt * P, (kt + 1) * P)
                        nc.tensor.matmul(L[:], kT[kvh][:, 0, ks], qq[:, 0],
                                         start=True, stop=False)
                        nc.tensor.matmul(L[:], kT[kvh][:, 1, ks], qq[:, 1],
                                         start=False, stop=True)
                        tt = spool.tile([P, QTILE], f32, tag="tt", name="tt")
                        nc.scalar.activation(tt[:], L[:], AF.Tanh, scale=QS)
                        pp = spool.tile([P, QTILE], f16, tag="pp", name="pp")
                        nc.scalar.activation(pp[:], tt[:], AF.Exp,
                                             bias=bias_e[:], scale=SOFT_CAP)
                        pu = pp[:]
                        if mi is not None:
                            pm = spool.tile([P, QTILE], f16, tag="pm",
                                            name="pm")
                            nc.vector.tensor_mul(pm[:], pp[:],
                                                 masks_sb[:, mi])
                            pu = pm[:]
                        for hc in range(2):
                            nc.tensor.matmul(
                                enc[hc][:],
                                v_all[:, kt, kvh * 2 * P + hc * P:
                                      kvh * 2 * P + (hc + 1) * P],
                                pu, start=st, stop=sp)
                        nc.tensor.matmul(s_ps[:], ones16[:], pu,
                                         start=st, stop=sp)
                        if sp:
                            qs = slice(qt * QTILE, (qt + 1) * QTILE)
                            rec = spool.tile([1, QTILE], f32, tag="rec",
                                             name="rec")
                            nc.vector.reciprocal_approx_fast(rec[:], s_ps[:])
                            rb = spool.tile([P, QTILE], f32, tag="rb",
                                            name="rb")
                            nc.gpsimd.partition_broadcast(rb[:], rec[:])
                            for hc in range(2):
                                nc.vector.tensor_mul(eT[qh][:, hc, qs],
                                                     enc[hc][:], rb[:])
                    return emit

                for i, (kt, mi) in enumerate(kts):
                    units.append(mk_tile(qh, kvh, enc, s_ps, kt, mi,
                                         i == 0, i == len(kts) - 1))
            return units

        pending = []

        def drain(k):
            for _ in range(min(k, len(pending))):
                pending.pop(0)()

        for n in range(NQT):
            ns = slice(n * QTILE, (n + 1) * QTILE)
            xts = load_x(n)
            for qh in range(HEADS_PER_CORE):
                qp = [psQ.tile([P, QTILE], f32, tag=f"qp{hc}",
                               name=f"qp{hc}") for hc in range(2)]
                for d in range(DC):
                    xt = xts[d // 2][:, d % 2]
                    for hc in range(2):
                        nc.tensor.matmul(
                            qp[hc][:], wq_sb[qh][:, d, hc * P:(hc + 1) * P],
                            xt, start=(d == 0), stop=(d == DC - 1))
                    if d % 2 == 1:
                        drain(1)
                rope_out(qp[0][:], qp[1][:], ns,
                         qT[n % 2][qh][:, 0], qT[n % 2][qh][:, 1])
                drain(1)
            drain(len(pending))
            pending = attn_units(n)

        # q weights no longer needed; free for out-proj weights
        psQ_cm.__exit__(None, None, None)
        wq_cm.__exit__(None, None, None)

        # ------------ P3: out projection, woven with attn(last block) ----
        ow_cm = tc.tile_pool(name="owp", bufs=1)       # 56KB
        owp = ow_cm.__enter__()
        o3_cm = tc.tile_pool(name="o3", bufs=2)        # 2KB
        o3pool = o3_cm.__enter__()
        po_cm = tc.tile_pool(name="po", bufs=2, space="PSUM")     # 2 banks
        popool = po_cm.__enter__()

        ow_sb = []
        for j in range(2 * HEADS_PER_CORE):
            wt = owp.tile([P, D], f16, tag=f"ow{j}", name=f"ow{j}")
            nc.sync.dma_start(wt[:], ow[j])
            ow_sb.append(wt)

        def p3_units():
            units = []
            for tci in range(T // P):
                ts_ = slice(tci * P, (tci + 1) * P)

                def mk_chunk(tci, ts_, nn):
                    def emit():
                        nns = slice(nn * QTILE, (nn + 1) * QTILE)
                        po = popool.tile([P, QTILE], f32, tag="po",
                                         name="po")
                        for j in range(2 * HEADS_PER_CORE):
                            nc.tensor.matmul(
                                po[:], eT[j // 2][:, j % 2, ts_],
                                ow_sb[j][:, nns],
                                start=(j == 0),
                                stop=(j == 2 * HEADS_PER_CORE - 1))
                        osb = o3pool.tile([P, QTILE], f16, tag="osb",
                                          name="osb")
                        if nn % 2 == 0:
                            nc.vector.tensor_copy(osb[:], po[:])
                        else:
                            nc.scalar.copy(osb[:], po[:])
                        nc.sync.dma_start(out[ts_, nns], osb[:])
                    return emit

                for nn in range(D // QTILE):
                    units.append(mk_chunk(tci, ts_, nn))
            return units

        # weave: final attention block and spans 0..2 of the out-proj can
        # interleave; spans for tci 12..15 need attn(3) complete anyway.
        p3u = p3_units()
        ready = [u for i, u in enumerate(p3u) if i < 12 * (D // QTILE)]
        tail = p3u[12 * (D // QTILE):]
        while pending or ready:
            drain(1)
            for _ in range(2):
                if ready:
                    ready.pop(0)()
        for u in tail:
            u()

        po_cm.__exit__(None, None, None)
        o3_cm.__exit__(None, None, None)
        ow_cm.__exit__(None, None, None)
        psL_cm.__exit__(None, None, None)
        psB_cm.__exit__(None, None, None)
        sp_cm.__exit__(None, None, None)
        rp_cm.__exit__(None, None, None)
        xp_cm.__exit__(None, None, None)
        per_cm.__exit__(None, None, None)

    nc.finalize()
    return nc


def _install_neff_cache():
    """Cache walrus-compiled NEFFs by BIR hash (compiles are minutes-long)."""
    import hashlib
    import shutil

    import concourse.bass2jax as b2j

    if getattr(b2j, "_ant_neff_cache_installed", False):
        return
    orig = b2j.compile_bir_kernel

    def cached(bir_json, tmpdir, neff_name="file.neff"):
        cdir = os.environ.get("NEFF_CACHE_DIR", "/tmp/neff_cache")
        os.makedirs(cdir, exist_ok=True)
        h = hashlib.sha256(bir_json).hexdigest()[:32]
        cpath = os.path.join(cdir, f"{h}.neff")
        if os.path.exists(cpath):
            dst = os.path.join(tmpdir, "sg00")
            os.makedirs(dst, exist_ok=True)
            dstf = os.path.join(dst, neff_name)
            shutil.copyfile(cpath, dstf)
            return dstf
        r = orig(bir_json, tmpdir, neff_name=neff_name)
        try:
            shutil.copyfile(r, cpath)
        except OSError:
            pass
        return r

    b2j.compile_bir_kernel = cached
    b2j._ant_neff_cache_installed = True


def kernel(x, segment_pos, attn_mask, q_w, kv_w, out_w):
    global LAST_RESULTS
    from concourse.bass_utils import run_bass_kernel_spmd

    _install_neff_cache()

    f16 = np.float16
    x = np.asarray(x, np.float32)
    segment_pos = np.asarray(segment_pos, np.int32)
    q_w = np.asarray(q_w, np.float32)
    kv_w = np.asarray(kv_w, np.float32)
    out_w = np.asarray(out_w, np.float32)

    # RoPE tables per batch, host layout [P, 2, T]: [cos, sin]
    ropes = []
    for b in range(B):
        pos = segment_pos[b].astype(np.float32)
        fraction = 2.0 * np.arange(P, dtype=np.float32) / H
        timescale = BASE_FREQ**fraction
        ang = pos[None, :] / timescale[:, None]          # [128, T]
        r = np.stack([np.cos(ang), np.sin(ang)]).astype(f16)
        ropes.append(np.ascontiguousarray(r.transpose(1, 0, 2)))
    masks = np.ascontiguousarray(
        _make_masks().transpose(1, 0, 2).astype(f16))

    def _wlayout(w):
        # [nh, D, Hc] -> [nh, P, DC, Hc]: per-partition contiguous spans
        return np.ascontiguousarray(
            w.reshape(-1, DC, P, w.shape[-1]).transpose(0, 2, 1, 3)
        ).astype(f16)

    key = "main"
    if key not in _NC_CACHE:
        _NC_CACHE[key] = _build_nc()
    nc = _NC_CACHE[key]

    in_maps = []
    for core in range(8):
        b, g = core // 4, core % 4
        # merged V weights: [D, 2H] with both kv heads side by side
        vw_m = np.concatenate(
            [kv_w[1, 2 * g], kv_w[1, 2 * g + 1]], axis=1)   # [D, 512]
        ow_l = np.ascontiguousarray(
            out_w[4 * g:4 * g + 4].reshape(4, 2, P, D).reshape(8, P, D)
        ).astype(f16)
        in_maps.append(
            {
                "xT": np.ascontiguousarray(x[b].T).astype(f16),
                "qw": _wlayout(q_w[4 * g: 4 * g + 4]),
                "kw": _wlayout(kv_w[0, 2 * g: 2 * g + 2]),
                "vw": _wlayout(vw_m[None])[0],
                "ow": ow_l,
                "rope": ropes[b],
                "msk": masks,
            }
        )

    res = run_bass_kernel_spmd(nc, in_maps, core_ids=list(range(8)))
    LAST_RESULTS = res

    outv = np.zeros((B, T, D), np.float32)
    for core in range(8):
        outv[core // 4] += res.results[core]["out"].astype(np.float32)
    return outv
